# revision 8
# baseline (speedup 1.0000x reference)
"""ConvexSH ColBERT loss kernel for 8 trn2 NeuronCores — v2.

Shards batch B=128 over 8 cores (16 rows each). Each core sees all NWAY=8
candidates for its rows; softmax + loss are core-local; the host averages the
8 partial sums.

v2 changes vs v1 (247µs -> target ~60µs):
- Host pre-swizzles doc into [n, p=k//2, b, (c d)] so each per-n doc block is
  one HWDGE dma_start with 16KB-contiguous partition lines (~97% DMA eff),
  replacing the SWDGE f32->bf16 cast DMA (11 GB/s/engine, 2µs fixed cost).
- f32->bf16 cast runs on the otherwise-idle GPSIMD engine, one big op per n.
- ssq/normalize are single big DVE instructions using stride-0 broadcast APs
  (the baseline's 256 small tensor_scalar ops paid ~200ns overhead each).
- mask/q/labels host-swizzled to their on-chip layouts; mask fold is one
  fused multiply (cast+mask) per n.
- softmax skips max-subtraction (scores bounded by |sum of 32 maxsims| <= 32,
  exp is safe in f32); label-only loss terms precomputed outside the tail.
- loss uses binary wmask identity: lv = t*(ln t_inv - ln p2) * (1-p)^wts.
"""

import sys
from contextlib import ExitStack

import numpy as np

for _p in ("/opt/trn_rl_repo", "/root/.axon_site/_ro/trn_rl_repo"):
    if _p not in sys.path:
        sys.path.append(_p)

import concourse.bacc as bacc
import concourse.bass as bass
import concourse.tile as tile
from concourse import mybir
from concourse.bass_utils import run_bass_kernel_spmd

AF = mybir.ActivationFunctionType
AX = mybir.AxisListType
ALU = mybir.AluOpType
F32 = mybir.dt.float32
BF16 = mybir.dt.bfloat16

NCORES = 8
B, LQ, LD, D, NWAY = 128, 32, 256, 128, 8
BS = B // NCORES  # 16 batch rows per core
NG = BS // 4      # 4 groups of 4 rows
ALPHA, GAMMA = 0.2, 2.0

TRACE = False
LAST_RESULTS = None


def _bc(ap, n):
    """Broadcast view: append a stride-0 dim of size n to an AP."""
    return bass.AP(ap.tensor, ap.offset, list(ap.ap) + [[0, n]])


def _build():
    nc = bacc.Bacc("TRN2", target_bir_lowering=False, detect_race_conditions=False)

    # host-swizzled inputs (see kernel() below)
    q_d = nc.dram_tensor("qn", [128, NG, D], F32, kind="ExternalInput")
    doc_d = nc.dram_tensor("doc", [NWAY, 128, BS, 2 * D], F32, kind="ExternalInput")
    mask_d = nc.dram_tensor("mask", [128, NWAY, BS, 2], F32, kind="ExternalInput")
    lab_d = nc.dram_tensor("lab", [4, NG, 3 * NWAY], F32, kind="ExternalInput")
    eye_d = nc.dram_tensor("eye", [128, 128], F32, kind="ExternalInput")
    y_d = nc.dram_tensor("y", [1, 1], F32, kind="ExternalOutput")

    with tile.TileContext(nc) as tc, ExitStack() as ctx:
        singles = ctx.enter_context(tc.tile_pool(name="singles", bufs=1))
        dpool = ctx.enter_context(tc.tile_pool(name="dpool", bufs=3))
        bpool = ctx.enter_context(tc.tile_pool(name="bpool", bufs=2))
        sqpool = ctx.enter_context(tc.tile_pool(name="sqpool", bufs=2))
        npool = ctx.enter_context(tc.tile_pool(name="npool", bufs=2))
        dtpool = ctx.enter_context(tc.tile_pool(name="dtpool", bufs=2))
        psT = ctx.enter_context(tc.tile_pool(name="psT", bufs=2, space="PSUM"))
        psMM = ctx.enter_context(tc.tile_pool(name="psMM", bufs=2, space="PSUM"))
        psS = ctx.enter_context(tc.tile_pool(name="psS", bufs=1, space="PSUM"))

        # ---- constants ----------------------------------------------------
        eye_f = singles.tile([128, 128], F32)
        nc.sync.dma_start(out=eye_f, in_=eye_d[:, :])
        eye_bf = singles.tile([128, 128], BF16)
        nc.vector.tensor_copy(eye_bf, eye_f)

        blockones = singles.tile([128, NG], F32)
        nc.vector.memset(blockones, 0.0)
        for m in range(4):
            nc.vector.memset(blockones[m * 32:(m + 1) * 32, m:m + 1], 1.0)
        ones4 = singles.tile([4, 1], F32)
        nc.vector.memset(ones4, 1.0)

        # ---- masks + labels (host-swizzled, straight DMA) -----------------
        mask_sb = singles.tile([128, NWAY, BS, 2], F32)
        nc.scalar.dma_start(out=mask_sb, in_=mask_d[:, :, :, :])
        lab_sb = singles.tile([4, NG, 3 * NWAY], F32)
        nc.scalar.dma_start(out=lab_sb, in_=lab_d[:, :, :])

        # ---- query: ssq + transpose --------------------------------------
        q_nat = singles.tile([128, NG, D], F32)  # p = (b%4)*32 + q
        nc.scalar.dma_start(out=q_nat, in_=q_d[:, :, :])

        qsq = sqpool.tile([128, D], F32, tag="sq")
        ssqq = singles.tile([128, NG], F32)
        for g in range(NG):
            nc.vector.scalar_tensor_tensor(
                out=qsq, in0=q_nat[:, g, :], scalar=1.0, in1=q_nat[:, g, :],
                op0=ALU.mult, op1=ALU.mult, accum_out=ssqq[:, g:g + 1])
        invq = singles.tile([128, NG], F32)
        nc.scalar.sqrt(invq, ssqq)
        nc.vector.reciprocal(invq, invq)

        psq = psT.tile([128, NG, D], F32, tag="psT")
        for g in range(NG):
            nc.tensor.transpose(psq[:, g, :], q_nat[:, g, :], eye_f)
        qT = singles.tile([128, NG, D], BF16)  # [d, g, (r*32+q)]
        nc.scalar.copy(qT.rearrange("p g t -> p (g t)"),
                       psq.rearrange("p g t -> p (g t)"))

        # ---- label-only loss precompute (off the critical tail) ----------
        t3 = lab_sb[:, :, 0:NWAY]
        r3 = lab_sb[:, :, NWAY:2 * NWAY]
        w3 = lab_sb[:, :, 2 * NWAY:3 * NWAY]

        def t32(name):
            return singles.tile([4, NG, NWAY], F32, tag=name, name=name)

        a3 = t32("a")    # 2w - 1
        b13 = t32("b1")  # 1 - w
        nc.vector.tensor_scalar(out=a3, in0=w3, scalar1=2.0, scalar2=-1.0,
                                op0=ALU.mult, op1=ALU.add)
        nc.vector.tensor_scalar(out=b13, in0=w3, scalar1=-1.0, scalar2=1.0,
                                op0=ALU.mult, op1=ALU.add)
        tinv = t32("tinv")
        nc.vector.tensor_mul(tinv, a3, t3)
        nc.vector.tensor_add(tinv, tinv, b13)
        tlti = t32("tlti")  # t * ln(t_inv)
        nc.scalar.activation(out=tlti, in_=tinv, func=AF.Ln)
        nc.vector.tensor_mul(tlti, tlti, t3)
        rr = t32("rr")
        nc.vector.reciprocal(rr, r3)
        wts = t32("wts")  # GAMMA - ALPHA/r + ALPHA/r0
        nc.vector.tensor_scalar(out=wts, in0=rr, scalar1=-ALPHA, scalar2=GAMMA,
                                op0=ALU.mult, op1=ALU.add)
        srr = singles.tile([4, NG, 1], F32)
        nc.vector.tensor_scalar_mul(srr, rr[:, :, 0:1], ALPHA)
        nc.vector.tensor_add(wts, wts, _bc(srr.rearrange("p g o -> p (g o)"), NWAY))

        # maxs[p = (b%4)*32+q, g, n]
        maxs = singles.tile([128, NG, NWAY], F32)

        # ---- main loop over candidates n ---------------------------------
        for n in range(NWAY):
            # doc block: p = k//2, free = (b, c, d); 16KB contiguous lines
            dn = dpool.tile([128, BS, 2 * D], F32, tag="dn")
            nc.sync.dma_start(out=dn, in_=doc_d[n])

            # squares on ACT (f32 in, bf16 out)
            sq = sqpool.tile([128, BS, 2, D], BF16, tag="sq")
            nc.scalar.activation(out=sq.rearrange("p b c d -> p (b c d)"),
                                 in_=dn.rearrange("p b e -> p (b e)"),
                                 func=AF.Square)
            # ssq per token: one 3D reduce (contiguous fast path)
            ssq = npool.tile([128, BS, 2], F32, tag="ssq")
            nc.vector.reduce_sum(out=ssq.rearrange("p b c -> p (b c)"),
                                 in_=sq.rearrange("p b c d -> p (b c) d"),
                                 axis=AX.X)

            # s = mask / sqrt(ssq)   (masked tokens -> exactly 0)
            srt = npool.tile([128, BS, 2], F32, tag="srt")
            nc.scalar.sqrt(srt.rearrange("p b c -> p (b c)"),
                           ssq.rearrange("p b c -> p (b c)"))
            nc.vector.reciprocal(srt.rearrange("p b c -> p (b c)"),
                                 srt.rearrange("p b c -> p (b c)"))
            sm = npool.tile([128, BS, 2], F32, tag="sm")
            nc.vector.tensor_mul(sm, srt, mask_sb[:, n, :, :])

            # normalize + cast in one op: bf16 out = f32 dn * bcast(s)
            dnb = bpool.tile([128, BS, 2, D], BF16, tag="dnb")
            nc.vector.tensor_mul(dnb, dn.rearrange("p b (c d) -> p b c d", c=2),
                                 _bc(sm, D))

            # per group g of 4 rows: 8 transposes -> 1 evac -> 4 matmuls
            psm = psMM.tile([128, NG, LD], F32, tag="psm")
            for g in range(NG):
                psd = psT.tile([128, 8, D], BF16, tag="psT")
                for r in range(4):
                    for c in range(2):
                        nc.tensor.transpose(psd[:, 2 * r + c, :],
                                            dnb[:, 4 * g + r, c, :], eye_bf)
                dt = dtpool.tile([128, 8, D], BF16, tag="dt")
                if g == 3:  # balance: one evacuation per n on DVE (2-port copy)
                    nc.vector.tensor_copy(dt.rearrange("p a d -> p (a d)"),
                                          psd.rearrange("p a d -> p (a d)"))
                else:
                    nc.scalar.copy(dt.rearrange("p a d -> p (a d)"),
                                   psd.rearrange("p a d -> p (a d)"))
                for r in range(4):
                    nc.tensor.matmul(psm[32 * r:32 * (r + 1), g, :],
                                     lhsT=qT[:, g, 32 * r:32 * (r + 1)],
                                     rhs=dt.rearrange("p a d -> p (a d)")[:, 2 * r * D:(2 * r + 2) * D],
                                     start=True, stop=True,
                                     tile_position=(0, 32 * r))
            # one 3D max over k for all 4 groups
            nc.vector.reduce_max(out=maxs[:, :, n:n + 1].rearrange("p g o -> p (g o)"),
                                 in_=psm, axis=AX.X)

        # ---- scores = colsum(maxs * invq) --------------------------------
        nc.vector.tensor_mul(maxs, maxs, _bc(invq, NWAY))
        scores_ps = psS.tile([4, NG * NWAY], F32, tag="scores")
        nc.tensor.matmul(scores_ps, lhsT=blockones,
                         rhs=maxs.rearrange("p g n -> p (g n)"),
                         start=True, stop=True)
        sc = singles.tile([4, NG, NWAY], F32)  # [m, g, n] = p for b = g*4+m
        nc.vector.tensor_copy(sc.rearrange("p g n -> p (g n)"), scores_ps)

        # ---- softmax over n (no max-subtraction; |scores| <= 32) ---------
        nc.scalar.activation(out=sc, in_=sc, func=AF.Exp)
        ssum = singles.tile([4, NG], F32)
        nc.vector.reduce_sum(out=ssum, in_=sc, axis=AX.X)
        nc.vector.reciprocal(ssum, ssum)
        nc.vector.tensor_mul(sc, sc, _bc(ssum, NWAY))
        # sc now holds p = softmax(scores)

        # ---- ConvexSH loss (binary wmask form) ---------------------------
        # p2 = a*p + b1 ; losses = t*ln(t_inv) - t*ln(p2) ; lv = losses*(1-p)^wts
        p2 = t32("p2")
        nc.vector.tensor_mul(p2, a3, sc)
        nc.vector.tensor_add(p2, p2, b13)
        lp2 = t32("lp2")
        nc.scalar.activation(out=lp2, in_=p2, func=AF.Ln)
        nc.vector.tensor_mul(lp2, lp2, t3)      # t * ln(p2)
        losses = t32("losses")
        nc.vector.tensor_sub(losses, tlti, lp2)
        omp = t32("omp")                        # 1 - p
        nc.vector.tensor_scalar(out=omp, in0=sc, scalar1=-1.0, scalar2=1.0,
                                op0=ALU.mult, op1=ALU.add)
        nc.scalar.activation(out=omp, in_=omp, func=AF.Ln)
        nc.vector.tensor_mul(omp, omp, wts)
        nc.scalar.activation(out=omp, in_=omp, func=AF.Exp)  # (1-p)^wts
        nc.vector.tensor_mul(losses, losses, omp)

        partial = singles.tile([4, 1], F32)
        nc.vector.reduce_sum(out=partial,
                             in_=losses.rearrange("p g n -> p (g n)"), axis=AX.X)
        out_ps = psS.tile([1, 1], F32, tag="out")
        nc.tensor.matmul(out_ps, lhsT=ones4, rhs=partial, start=True, stop=True)
        out_sb = singles.tile([1, 1], F32)
        nc.vector.tensor_copy(out_sb, out_ps)
        nc.sync.dma_start(out=y_d[:, :], in_=out_sb)

    nc.finalize()
    return nc


_nc_cache = None


def kernel(query_reps, doc_reps, doc_masks, labels):
    global _nc_cache, LAST_RESULTS
    if _nc_cache is None:
        _nc_cache = _build()
    nc = _nc_cache

    query_reps = np.asarray(query_reps, dtype=np.float32)
    doc_reps = np.asarray(doc_reps, dtype=np.float32)
    doc_masks = np.asarray(doc_masks, dtype=np.float32)
    labels = np.asarray(labels, dtype=np.float32)

    eye = np.eye(128, dtype=np.float32)
    in_maps = []
    for core in range(NCORES):
        sl = slice(core * BS, (core + 1) * BS)
        # doc[n, b, k, d] -> [n, p=k//2, b, (c=k%2, d)]
        doc = doc_reps[:, sl].reshape(NWAY, BS, 128, 2, D)
        doc = np.ascontiguousarray(doc.transpose(0, 2, 1, 3, 4)).reshape(
            NWAY, 128, BS, 2 * D)
        # mask[n, b, k] -> [p, n, b, c]
        msk = doc_masks[:, sl].reshape(NWAY, BS, 128, 2)
        msk = np.ascontiguousarray(msk.transpose(2, 0, 1, 3))
        # q[b, q, d] -> [(r*32+q), g, d]  for b = g*4 + r
        qn = query_reps[sl].reshape(NG, 4, LQ, D)
        qn = np.ascontiguousarray(qn.transpose(1, 2, 0, 3)).reshape(128, NG, D)
        # labels[b, c] -> [m, g, c]  for b = g*4 + m
        lab = labels[sl].reshape(NG, 4, 3 * NWAY)
        lab = np.ascontiguousarray(lab.transpose(1, 0, 2))
        in_maps.append({"qn": qn, "doc": doc, "mask": msk, "lab": lab, "eye": eye})

    kwargs = {}
    if TRACE:
        kwargs["trace"] = True
    res = run_bass_kernel_spmd(nc, in_maps, core_ids=list(range(NCORES)), **kwargs)
    LAST_RESULTS = res
    total = sum(float(res.results[c]["y"][0, 0]) for c in range(NCORES))
    return np.array(total / (B * NWAY), dtype=np.float32)
